# revision 1
# baseline (speedup 1.0000x reference)
"""DeepSpeed MoE dense-MLP kernel for Trainium2 (8 NeuronCores, SPMD).

Strategy: data-parallel over tokens (B*S = 4096 tokens -> 512/core).
Each core computes its tokens' full MLP:
    h  = gelu_tanh(x @ W1 + b1)       [512, 16384]
    out = h @ W2 + b2                 [512, 4096]
All matmuls in bf16 with fp32 PSUM accumulation; second-level
accumulation (over f-chunks) in fp32 SBUF.  No collectives.

Host-side prep (inside kernel()): cast to bf16, transpose x, relayout
W1/W2 so every DMA is fully contiguous per partition.
"""

import numpy as np
import ml_dtypes

import concourse.bass as bass
import concourse.bacc as bacc
import concourse.tile as tile
import concourse.mybir as mybir
from concourse.bass_utils import run_bass_kernel_spmd

BF16 = ml_dtypes.bfloat16
FP32 = np.float32

N_CORES = 8


def build_nc(M, H, F, f_chunk=8, hb=512, n_cores=N_CORES):
    """Emit the per-core kernel.  M = tokens per core."""
    P = 128
    KT = H // P          # k-tiles (contraction of matmul 1)
    FT = F // P          # f-tiles
    MS = M // P          # m sub-tiles
    NHB = H // hb        # h blocks (output columns of matmul 2)
    NFC = FT // f_chunk  # phase-2 chunks
    assert M <= 512  # moving free-dim limit per matmul

    dt = mybir.dt
    nc = bacc.Bacc("TRN2", target_bir_lowering=False, debug=False,
                   num_devices=n_cores)

    # DRAM I/O (per core).
    xT_d = nc.dram_tensor("xt", [H, M], dt.bfloat16, kind="ExternalInput").ap()
    # w1l[ft, p, k*P+m] = W1[k*P+p, ft*P+m]  (lhsT tiles, contiguous per ft)
    w1_d = nc.dram_tensor("w1l", [FT, P, KT * P], dt.bfloat16,
                          kind="ExternalInput").ap()
    # w2l[hb, ft, p, n] = W2[ft*P+p, hb*HB+n]  (rhs tiles, contiguous)
    w2_d = nc.dram_tensor("w2l", [NHB, FT, P, hb], dt.bfloat16,
                          kind="ExternalInput").ap()
    b1_d = nc.dram_tensor("b1t", [P, FT], dt.float32, kind="ExternalInput").ap()
    b2_d = nc.dram_tensor("b2", [1, H], dt.float32, kind="ExternalInput").ap()
    out_d = nc.dram_tensor("out", [M, H], dt.float32, kind="ExternalOutput").ap()

    with tile.TileContext(nc) as tc:
        with (
            tc.tile_pool(name="xt", bufs=KT) as xt_pool,
            tc.tile_pool(name="w1", bufs=3) as w1_pool,
            tc.tile_pool(name="w2", bufs=2 * f_chunk) as w2_pool,
            tc.tile_pool(name="ht", bufs=2 * f_chunk) as ht_pool,
            tc.tile_pool(name="o2", bufs=MS) as o2_pool,
            tc.tile_pool(name="cst", bufs=1) as cst_pool,
            tc.tile_pool(name="ps1", bufs=2, space=bass.MemorySpace.PSUM) as ps1,
            tc.tile_pool(name="ps2", bufs=2, space=bass.MemorySpace.PSUM) as ps2,
        ):
            # Resident tensors ------------------------------------------------
            b1_s = cst_pool.tile([P, FT], dt.float32, tag="b1")
            nc.sync.dma_start(b1_s[:], b1_d)

            xt_s = []
            for k in range(KT):
                t = xt_pool.tile([P, M], dt.bfloat16, tag="xt")
                nc.sync.dma_start(t[:], xT_d[k * P:(k + 1) * P, :])
                xt_s.append(t)

            # out accumulators, initialised with the broadcast output bias
            o2_s = []
            for ms in range(MS):
                t = o2_pool.tile([P, H], dt.float32, tag="o2")
                nc.sync.dma_start(t[:], b2_d.partition_broadcast(P))
                o2_s.append(t)

            # Main pipeline ---------------------------------------------------
            for fc in range(NFC):
                # phase 1: h^T tiles for this chunk
                ht_tiles = []
                for fi in range(f_chunk):
                    ft = fc * f_chunk + fi
                    w1_s = w1_pool.tile([P, KT * P], dt.bfloat16, tag="w1")
                    nc.sync.dma_start(w1_s[:], w1_d[ft])
                    acc = ps1.tile([P, M], dt.float32, tag="ps1")
                    for k in range(KT):
                        nc.tensor.matmul(
                            acc[:],
                            w1_s[:, k * P:(k + 1) * P],
                            xt_s[k][:],
                            start=(k == 0),
                            stop=(k == KT - 1),
                        )
                    ht = ht_pool.tile([P, M], dt.bfloat16, tag="ht")
                    nc.scalar.activation(
                        ht[:], acc[:],
                        mybir.ActivationFunctionType.Gelu_apprx_tanh,
                        bias=b1_s[:, ft:ft + 1],
                    )
                    ht_tiles.append(ht)

                # phase 2: accumulate this chunk's contribution to out
                for hbi in range(NHB):
                    w2_tiles = []
                    for fi in range(f_chunk):
                        ft = fc * f_chunk + fi
                        w2_s = w2_pool.tile([P, hb], dt.bfloat16, tag="w2")
                        nc.sync.dma_start(w2_s[:], w2_d[hbi, ft])
                        w2_tiles.append(w2_s)
                    for ms in range(MS):
                        acc2 = ps2.tile([P, hb], dt.float32, tag="ps2")
                        for fi in range(f_chunk):
                            nc.tensor.matmul(
                                acc2[:],
                                ht_tiles[fi][:, ms * P:(ms + 1) * P],
                                w2_tiles[fi][:],
                                start=(fi == 0),
                                stop=(fi == f_chunk - 1),
                            )
                        dst = o2_s[ms][:, hbi * hb:(hbi + 1) * hb]
                        nc.vector.tensor_add(dst, dst, acc2[:])

            # Store -----------------------------------------------------------
            for ms in range(MS):
                nc.sync.dma_start(out_d[ms * P:(ms + 1) * P, :], o2_s[ms][:])

    nc.compile()
    return nc


def prep_inputs(x, inter_w, inter_b, output_w, output_b, n_cores=N_CORES,
                hb=512):
    """Host-side shard + relayout.  Returns per-core input maps."""
    P = 128
    H = x.shape[-1]
    F = inter_w.shape[1]
    KT, FT, NHB = H // P, F // P, H // hb
    tokens = int(np.prod(x.shape[:-1]))
    M = tokens // n_cores

    xT = np.ascontiguousarray(x.reshape(tokens, H).T.astype(BF16))  # [H, tokens]
    w1l = np.ascontiguousarray(
        inter_w.astype(BF16).reshape(KT, P, FT, P).transpose(2, 1, 0, 3)
    ).reshape(FT, P, KT * P)
    w2l = np.ascontiguousarray(
        output_w.astype(BF16).reshape(FT, P, NHB, hb).transpose(2, 0, 1, 3)
    )
    b1t = np.ascontiguousarray(
        inter_b.astype(FP32).reshape(FT, P).T
    )
    b2 = output_b.astype(FP32).reshape(1, H)

    in_maps = []
    for c in range(n_cores):
        in_maps.append({
            "xt": np.ascontiguousarray(xT[:, c * M:(c + 1) * M]),
            "w1l": w1l,
            "w2l": w2l,
            "b1t": b1t,
            "b2": b2,
        })
    return in_maps


_NC_CACHE = {}


def _get_nc(M, H, F):
    key = (M, H, F)
    if key not in _NC_CACHE:
        _NC_CACHE[key] = build_nc(M, H, F)
    return _NC_CACHE[key]


def run(x, inter_w, inter_b, output_w, output_b, trace=False):
    tokens = int(np.prod(x.shape[:-1]))
    H = x.shape[-1]
    F = inter_w.shape[1]
    M = tokens // N_CORES
    nc = _get_nc(M, H, F)
    in_maps = prep_inputs(x, inter_w, inter_b, output_w, output_b)
    res = run_bass_kernel_spmd(nc, in_maps, list(range(N_CORES)), trace=trace)
    out = np.concatenate([res.results[c]["out"] for c in range(N_CORES)], axis=0)
    return out.reshape(x.shape), res


def kernel(x, inter_w, inter_b, output_w, output_b):
    out, _ = run(np.asarray(x), np.asarray(inter_w), np.asarray(inter_b),
                 np.asarray(output_w), np.asarray(output_b))
    return out


# revision 12
# speedup vs baseline: 24.5267x; 24.5267x over previous
"""DeepSpeed MoE dense-MLP kernel for Trainium2 (8 NeuronCores, SPMD).

Strategy: data-parallel over tokens (B*S = 4096 tokens -> 512/core).
Each core computes its tokens' full MLP:
    h  = gelu_tanh(x @ W1 + b1)       [512, 16384]
    out = h @ W2 + b2                 [512, 4096]
All matmuls in bf16 with fp32 PSUM accumulation; second-level
accumulation (over f-chunks) in fp32 SBUF.  No collectives.

Host-side prep (inside kernel()): cast to bf16, transpose x, relayout
W1/W2 so every DMA is fully contiguous per partition.
"""

import numpy as np
import ml_dtypes

import concourse.bass as bass
import concourse.bacc as bacc
import concourse.tile as tile
import concourse.mybir as mybir
from concourse.bass_utils import run_bass_kernel_spmd

BF16 = ml_dtypes.bfloat16
FP32 = np.float32

N_CORES = 8


def build_nc(M, H, F, f_chunk=8, hb=512, n_cores=N_CORES, reps=1):
    """Emit the per-core kernel.  M = tokens per core.  reps>1 repeats the
    whole compute body (for overhead-cancelling HW timing)."""
    P = 128
    KT = H // P          # k-tiles (contraction of matmul 1)
    FT = F // P          # f-tiles
    MS = M // P          # m sub-tiles
    NHB = H // hb        # h blocks (output columns of matmul 2)
    NFC = FT // f_chunk  # phase-2 chunks
    assert M <= 512  # moving free-dim limit per matmul

    dt = mybir.dt
    nc = bacc.Bacc("TRN2", target_bir_lowering=False, debug=False,
                   num_devices=n_cores)

    # DRAM I/O (per core).
    xT_d = nc.dram_tensor("xt", [H, M], dt.bfloat16, kind="ExternalInput").ap()
    # w1l[ft, p, k*P+m] = W1[k*P+p, ft*P+m]  (lhsT tiles, contiguous per ft)
    w1_d = nc.dram_tensor("w1l", [FT, P, KT * P], dt.bfloat16,
                          kind="ExternalInput").ap()
    # w2l[hb, ft, p, n] = W2[ft*P+p, hb*HB+n]  (rhs tiles, contiguous)
    w2_d = nc.dram_tensor("w2l", [NHB, FT, P, hb], dt.bfloat16,
                          kind="ExternalInput").ap()
    b1_d = nc.dram_tensor("b1t", [P, FT], dt.float32, kind="ExternalInput").ap()
    b2_d = nc.dram_tensor("b2", [1, H], dt.float32, kind="ExternalInput").ap()
    out_d = nc.dram_tensor("out", [M, H], dt.float32, kind="ExternalOutput").ap()

    with tile.TileContext(nc) as tc:
        with (
            tc.tile_pool(name="xt", bufs=KT) as xt_pool,
            tc.tile_pool(name="w1", bufs=4) as w1_pool,
            tc.tile_pool(name="w2", bufs=2 * f_chunk) as w2_pool,
            tc.tile_pool(name="ht", bufs=2 * f_chunk) as ht_pool,
            tc.tile_pool(name="o2", bufs=MS) as o2_pool,
            tc.tile_pool(name="cst", bufs=1) as cst_pool,
            tc.tile_pool(name="ps1", bufs=3, space=bass.MemorySpace.PSUM) as ps1,
            tc.tile_pool(name="ps2", bufs=3, space=bass.MemorySpace.PSUM) as ps2,
        ):
            # Resident tensors ------------------------------------------------
            # first weight tile up front so PE can start ASAP
            w1_first = w1_pool.tile([P, KT * P], dt.bfloat16, tag="w1")
            nc.sync.dma_start(w1_first[:], w1_d[0])

            b1_s = cst_pool.tile([P, FT], dt.float32, tag="b1")
            nc.sync.dma_start(b1_s[:], b1_d)

            xt_s = []
            for k in range(KT):
                t = xt_pool.tile([P, M], dt.bfloat16, tag="xt")
                nc.sync.dma_start(t[:], xT_d[k * P:(k + 1) * P, :])
                xt_s.append(t)

            first_w1 = [w1_first]

            def body():
                o2_s = []

                for fc in range(NFC):
                    # phase 1: h^T tiles for this chunk
                    ht_tiles = []
                    for fi in range(f_chunk):
                        ft = fc * f_chunk + fi
                        if ft == 0 and first_w1[0] is not None:
                            w1_s = first_w1[0]
                            first_w1[0] = None
                        else:
                            w1_s = w1_pool.tile([P, KT * P], dt.bfloat16,
                                                tag="w1")
                            nc.sync.dma_start(w1_s[:], w1_d[ft])
                        acc = ps1.tile([P, M], dt.float32, tag="ps1")
                        for k in range(KT):
                            nc.tensor.matmul(
                                acc[:],
                                w1_s[:, k * P:(k + 1) * P],
                                xt_s[k][:],
                                start=(k == 0),
                                stop=(k == KT - 1),
                            )
                        ht = ht_pool.tile([P, M], dt.bfloat16, tag="ht")
                        nc.scalar.activation(
                            ht[:], acc[:],
                            mybir.ActivationFunctionType.Gelu_apprx_tanh,
                            bias=b1_s[:, ft:ft + 1],
                        )
                        ht_tiles.append(ht)

                    if fc == 0:
                        # out accumulators, initialised with the broadcast
                        # output bias (emitted late so these DMAs don't
                        # delay the startup weight loads)
                        for ms in range(MS):
                            t = o2_pool.tile([P, H], dt.float32, tag="o2")
                            nc.sync.dma_start(t[:], b2_d.partition_broadcast(P))
                            o2_s.append(t)

                    # phase 2: accumulate this chunk's contribution to out
                    for hbi in range(NHB):
                        w2_tiles = []
                        for fi in range(f_chunk):
                            ft = fc * f_chunk + fi
                            w2_s = w2_pool.tile([P, hb], dt.bfloat16, tag="w2")
                            nc.sync.dma_start(w2_s[:], w2_d[hbi, ft])
                            w2_tiles.append(w2_s)
                        for ms in range(MS):
                            acc2 = ps2.tile([P, hb], dt.float32, tag="ps2")
                            for fi in range(f_chunk):
                                nc.tensor.matmul(
                                    acc2[:],
                                    ht_tiles[fi][:, ms * P:(ms + 1) * P],
                                    w2_tiles[fi][:],
                                    start=(fi == 0),
                                    stop=(fi == f_chunk - 1),
                                )
                            dst = o2_s[ms][:, hbi * hb:(hbi + 1) * hb]
                            nc.vector.tensor_add(dst, dst, acc2[:])
                            if fc == NFC - 1:
                                # store each output block as soon as its
                                # last accumulation lands
                                nc.sync.dma_start(
                                    out_d[ms * P:(ms + 1) * P,
                                          hbi * hb:(hbi + 1) * hb],
                                    dst)

            for _rep in range(reps):
                body()

    nc.compile()
    return nc


def prep_inputs(x, inter_w, inter_b, output_w, output_b, n_cores=N_CORES,
                hb=512):
    """Host-side shard + relayout.  Returns per-core input maps."""
    P = 128
    H = x.shape[-1]
    F = inter_w.shape[1]
    KT, FT, NHB = H // P, F // P, H // hb
    tokens = int(np.prod(x.shape[:-1]))
    M = tokens // n_cores

    xT = np.ascontiguousarray(x.reshape(tokens, H).T.astype(BF16))  # [H, tokens]
    w1l = np.ascontiguousarray(
        inter_w.astype(BF16).reshape(KT, P, FT, P).transpose(2, 1, 0, 3)
    ).reshape(FT, P, KT * P)
    w2l = np.ascontiguousarray(
        output_w.astype(BF16).reshape(FT, P, NHB, hb).transpose(2, 0, 1, 3)
    )
    b1t = np.ascontiguousarray(
        inter_b.astype(FP32).reshape(FT, P).T
    )
    b2 = output_b.astype(FP32).reshape(1, H)

    in_maps = []
    for c in range(n_cores):
        in_maps.append({
            "xt": np.ascontiguousarray(xT[:, c * M:(c + 1) * M]),
            "w1l": w1l,
            "w2l": w2l,
            "b1t": b1t,
            "b2": b2,
        })
    return in_maps


_NC_CACHE = {}


def _get_nc(M, H, F):
    key = (M, H, F)
    if key not in _NC_CACHE:
        _NC_CACHE[key] = build_nc(M, H, F)
    return _NC_CACHE[key]


def run(x, inter_w, inter_b, output_w, output_b, trace=False):
    tokens = int(np.prod(x.shape[:-1]))
    H = x.shape[-1]
    F = inter_w.shape[1]
    M = tokens // N_CORES
    nc = _get_nc(M, H, F)
    in_maps = prep_inputs(x, inter_w, inter_b, output_w, output_b)
    res = run_bass_kernel_spmd(nc, in_maps, list(range(N_CORES)), trace=trace)
    out = np.concatenate([res.results[c]["out"] for c in range(N_CORES)], axis=0)
    return out.reshape(x.shape), res


def kernel(x, inter_w, inter_b, output_w, output_b):
    out, _ = run(np.asarray(x), np.asarray(inter_w), np.asarray(inter_b),
                 np.asarray(output_w), np.asarray(output_b))
    return out
